# revision 3
# baseline (speedup 1.0000x reference)
"""FFNN-Transducer joint-lattice kernel for 8 Trainium2 NeuronCores.

Data-parallel over batch B=8 (one sample per core). The device computes the
dense T x (U+1) joint lattice:
    out[t,u,:] = tanh(enc_proj[t,:] + pred_bias[u,:]) @ jw2
where enc_proj = enc @ jw1[:E] is computed on-device and pred_bias[u,:]
(= pred @ jw1[E:] + jb1) comes from the tiny prediction network (done on host,
<0.3% of total FLOPs). jb2-add and the ragged masking are host epilogues.

TRN2 fp32 matmul runs at 1/4 rate (hi/lo 2-pass), so all TensorE-facing
tensors are fp16; PSUM accumulation stays fp32; the output is stored fp16
(host epilogue upcasts) to halve the HBM store traffic.

Device pipeline per core, in t-blocks of 128 (T padded to 1024):
  PE:   enc_proj prologue (4-acc matmuls per 128-t chunk);
        per 4-t chunk one "selection" matmul materializing
        A[j,(u,t)] = enc_proj[t,j] + bias[u,j] in PSUM, u-major
        (lhsT = [16 enc rows ; 101 bias rows], rhs = 0/1 selection);
        per u one [128x128] x [128x88] joint matmul into PSUM; the
        u-major hid layout makes the joint lhsT slices contiguous so
        the compiler's fast-weight-load path stays enabled.
  ACT:  batched tanh PSUM->SBUF fp16 into the u-major hid layout.
  DVE:  PSUM->SBUF staging evacuation into [t-partition, (u,v)] fp16.
  DMA:  enc_proj is bounced through DRAM (2 large descriptors) into the
        117-row "combined" lhsT tensor -- replaces 64 tiny SBUF->SBUF
        transfers that were DMA-issue bound; per-block stores
        [128, 8888] fp16 with contiguous 17.8KB partition lines.
"""

import os
import sys

for _p in ("/opt/trn_rl_repo", "/root/.axon_site/_ro/trn_rl_repo"):
    if os.path.isdir(_p) and _p not in sys.path:
        sys.path.append(_p)

import numpy as np

import concourse.bass as bass
import concourse.tile as tile
from concourse import bacc, mybir
from concourse.bass_utils import run_bass_kernel_spmd

# Problem dims (hardcoded per contract)
B, T, E = 8, 1000, 512
U = 100
U1 = U + 1          # 101 joint positions
H, D, P = 2, 256, 256
J, V = 128, 88
BLANK = V - 1
N_CORES = 8

# Device tiling
TP = 1024           # padded T (8 blocks of 128)
TB = 128            # t-steps per block (= joint-matmul lhsT cols)
NB = TP // TB       # 8 blocks
HALF = 8            # t-steps per A-PSUM tile ([128, 1024] = 2 banks)
CH = 4              # t-steps per pre-add matmul chunk (N = 404)
SPAN = 16           # t-steps per combined lhsT slice (K = SPAN + U1 = 117)
NSPAN = TP // SPAN  # 64 spans
UG = 10             # u-steps per M-PSUM tile ([128, 1024] = 2 banks)
NUG = 11            # u-groups per block (10 full + 1 leftover)

F32 = mybir.dt.float32
F16 = mybir.dt.float16

_CACHE = {}


def _build_program(reps=1):
    nc = bacc.Bacc("TRN2", target_bir_lowering=False, debug=False)

    encT = nc.dram_tensor("encT", [E, TP], F16, kind="ExternalInput").ap()
    jw1enc = nc.dram_tensor("jw1enc", [E, J], F16, kind="ExternalInput").ap()
    jw2d = nc.dram_tensor("jw2d", [J, V], F16, kind="ExternalInput").ap()
    biasu = nc.dram_tensor("biasu", [U1, J], F16, kind="ExternalInput").ap()
    seld = nc.dram_tensor("seld", [SPAN + U1, SPAN * U1], F16, kind="ExternalInput").ap()
    epd = nc.dram_tensor("epd", [128, NB * J], F16, kind="Internal").ap()
    out = nc.dram_tensor("out", [T, U1 * V], F16, kind="ExternalOutput").ap()

    with tile.TileContext(nc) as tc:
        with (
            tc.tile_pool(name="singles", bufs=1) as singles,
            tc.tile_pool(name="hidp", bufs=3) as hidp,
            tc.tile_pool(name="stgp", bufs=2) as stgp,
            tc.tile_pool(name="psA", bufs=2, space="PSUM") as psA,
            tc.tile_pool(name="psM", bufs=2, space="PSUM") as psM,
        ):
            # ---- persistent SBUF tensors ----
            encT_sb = []
            for k in range(4):
                t_ = singles.tile([128, TP], F16, tag=f"encT{k}")
                nc.sync.dma_start(out=t_[:, :], in_=encT[k * 128:(k + 1) * 128, :])
                encT_sb.append(t_)
            jw1_sb = []
            for k in range(4):
                t_ = singles.tile([128, J], F16, tag=f"jw1_{k}")
                nc.sync.dma_start(out=t_[:, :], in_=jw1enc[k * 128:(k + 1) * 128, :])
                jw1_sb.append(t_)
            jw2_sb = singles.tile([J, V], F16, tag="jw2")
            nc.sync.dma_start(out=jw2_sb[:, :], in_=jw2d[:, :])
            sel_sb = singles.tile([SPAN + U1, SPAN * U1], F16, tag="sel")
            nc.sync.dma_start(out=sel_sb[:, :], in_=seld[:, :])
            # combined lhsT tensors (double-buffered across reps):
            # rows 0:16 = per-span enc_proj slices (refreshed per rep via the
            # DRAM bounce), rows 16:117 = bias rows replicated per span
            # (built once by doubling copies).
            combined = []
            for p_ in range(2):
                t_ = singles.tile([SPAN + U1, NSPAN * J], F16, tag=f"comb{p_}")
                nc.sync.dma_start(out=t_[SPAN:SPAN + U1, 0:J], in_=biasu[:, :])
                w = J
                while w < NSPAN * J:
                    # one-time doubling replication; DMA because engines can't
                    # address a 101-partition window starting at partition 16
                    nc.sync.dma_start(
                        out=t_[SPAN:SPAN + U1, w:2 * w],
                        in_=t_[SPAN:SPAN + U1, 0:w],
                    )
                    w *= 2
                combined.append(t_)
            # enc_proj row-major: [t % 128, (t//128)*J + j] (block-aligned)
            enc_proj = singles.tile([128, NB * J], F16, tag="encproj")

            for rep in range(reps):
                _emit_rep(nc, hidp, stgp, psA, psM,
                          encT_sb, jw1_sb, jw2_sb, sel_sb,
                          combined[rep % 2], enc_proj, epd,
                          out, rep)

    nc.compile()
    return nc


def _emit_rep(nc, hidp, stgp, psA, psM,
              encT_sb, jw1_sb, jw2_sb, sel_sb, comb, enc_proj, epd, out, rep):
    # ---- prologue: enc_proj[t, j] = sum_e enc[t, e] * jw1enc[e, j] ----
    for cb in range(NB):
        ep = psA.tile([TB, J], F32, tag="A", name=f"ep{rep}_{cb}")
        for k in range(4):
            nc.tensor.matmul(
                ep[:, :],
                encT_sb[k][:, cb * TB:(cb + 1) * TB],
                jw1_sb[k][:, :],
                start=(k == 0),
                stop=(k == 3),
            )
        nc.vector.tensor_copy(out=enc_proj[:, cb * J:(cb + 1) * J], in_=ep[:, :])
    # bounce enc_proj through DRAM to land the per-span 16-row slices on
    # partitions 0:16 of `comb` (2 large DMAs instead of 64 tiny ones).
    nc.sync.dma_start(out=epd[:, :], in_=enc_proj[:, :])
    # comb[0:16] col layout: (cb, s, j) -> (cb*8+s)*J + j; epd flat offset for
    # (r, cb, s, j) = (s*16+r)*1024 + cb*128 + j
    src = epd.rearrange("(s r) (cb j) -> r cb s j", s=8, cb=NB)
    dst = comb[0:SPAN, :].rearrange("r (cb s j) -> r cb s j", cb=NB, s=8)
    nc.sync.dma_start(out=dst, in_=src)

    hid_tiles = [None] * NB
    stg_tiles = [None] * NB

    def front(b, step):
        # pre-add matmuls + tanh for the 8-t half (b, step); u-major hid
        t0 = b * TB + step * HALF
        if step == 0:
            hid_tiles[b] = hidp.tile([128, U1 * TB], F16, tag="hid",
                                     name=f"hid{rep}_{b}")
        hid2 = hid_tiles[b]
        sp = t0 // SPAN
        cb_t = comb[:, sp * J:(sp + 1) * J]
        A = psA.tile([128, 1024], F32, tag="A", name=f"A{rep}_{b}_{step}")
        lh = (t0 % SPAN) // HALF  # 0 or 1: which pair of chunks in sel
        for c in range(2):
            nc.tensor.matmul(
                A[:, c * 512:c * 512 + CH * U1],
                cb_t,
                sel_sb[:, (lh * 2 + c) * CH * U1:(lh * 2 + c + 1) * CH * U1],
                start=True,
                stop=True,
            )
        # tanh into u-major hid: col = u*TB + t
        base = step * HALF
        out4 = (hid2.rearrange("p (u t) -> p u t", t=TB)[:, :, base:base + HALF]
                .rearrange("p u (c tl) -> p c u tl", c=2))
        in4 = (A.rearrange("p (c x) -> p c x", c=2)[:, :, 0:CH * U1]
               .rearrange("p c (u tl) -> p c u tl", tl=CH))
        nc.scalar.activation(
            out=out4,
            in_=in4,
            func=mybir.ActivationFunctionType.Tanh,
        )

    def back(b, ug):
        # joint matmuls + evacuation for u-group ug of block b
        hid2 = hid_tiles[b]
        if ug == 0:
            stg_tiles[b] = stgp.tile([TB, U1 * V], F16, tag="stg",
                                     name=f"stg{rep}_{b}")
        stg = stg_tiles[b]
        u0 = ug * UG
        n_u = UG if ug < NUG - 1 else U1 - u0
        M = psM.tile([TB, 1024], F32, tag="M", name=f"M{rep}_{b}_{ug}")
        for i in range(n_u):
            col = (i // 5) * 512 + (i % 5) * V
            nc.tensor.matmul(
                M[:, col:col + V],
                hid2[:, (u0 + i) * TB:(u0 + i + 1) * TB],
                jw2_sb[:, :],
                start=True,
                stop=True,
            )
        if n_u == UG:
            nc.vector.tensor_copy(
                out=stg[:, u0 * V:(u0 + UG) * V].rearrange("p (bk x) -> p bk x", bk=2),
                in_=M.rearrange("p (bk x) -> p bk x", bk=2)[:, :, 0:5 * V],
            )
        else:
            nc.vector.tensor_copy(
                out=stg[:, u0 * V:(u0 + n_u) * V],
                in_=M[:, 0:n_u * V],
            )
        if ug == NUG - 1:
            n_t = min(TB, T - b * TB)
            nc.sync.dma_start(
                out=out[b * TB:b * TB + n_t, :],
                in_=stg[0:n_t, :],
            )

    # software-pipelined emission: block b's fronts interleave with b-1's backs
    for b in range(NB):
        for step in range(TB // HALF):  # 16
            front(b, step)
            if b >= 1 and step < NUG:
                back(b - 1, step)
    for ug in range(NUG):
        back(NB - 1, ug)


def _host_pred_bias(targets_b, emb, pw1, pb1, pw2, pb2, jw1, jb1):
    """bias[u, j] = (pred @ jw1[E:] + jb1)[u, j] for the 101 joint positions."""
    ext = np.concatenate([np.full(H, BLANK, np.int64), targets_b.astype(np.int64)])
    e = np.concatenate([emb[ext[1:U1 + 1]], emb[ext[0:U1]]], axis=1)  # [101, 512]
    h = np.tanh(e @ pw1 + pb1)
    pred = np.tanh(h @ pw2 + pb2)
    return (pred @ jw1[E:] + jb1).astype(np.float32)  # [101, 128]


def _make_sel():
    # u-major selection: chunk ch covers t-locals [ch*CH, (ch+1)*CH); within a
    # chunk, column (u*CH + tl) selects enc row (ch*CH+tl) + bias row u.
    sel = np.zeros((SPAN + U1, SPAN * U1), np.float16)
    for ch in range(SPAN // CH):
        colbase = ch * CH * U1
        for tl in range(CH):
            g = ch * CH + tl
            sel[g, colbase + tl:colbase + CH * U1:CH] = 1.0
        for u in range(U1):
            sel[SPAN + u, colbase + u * CH:colbase + u * CH + CH] += 1.0
    return sel


def _make_in_maps(encoder_states, targets, emb, pw1, pb1, pw2, pb2, jw1, jb1, jw2):
    encoder_states = np.asarray(encoder_states, dtype=np.float32)
    jw1 = np.asarray(jw1, dtype=np.float32)
    jw2_np = np.ascontiguousarray(np.asarray(jw2, dtype=np.float32)).astype(np.float16)
    jw1enc = np.ascontiguousarray(jw1[:E]).astype(np.float16)
    sel = _make_sel()

    in_maps = []
    for b in range(B):
        encT_b = np.zeros((E, TP), np.float16)
        encT_b[:, :T] = encoder_states[b].T.astype(np.float16)
        bias_b = _host_pred_bias(
            np.asarray(targets[b]), np.asarray(emb, np.float32),
            np.asarray(pw1, np.float32), np.asarray(pb1, np.float32),
            np.asarray(pw2, np.float32), np.asarray(pb2, np.float32),
            jw1, np.asarray(jb1, np.float32),
        ).astype(np.float16)
        in_maps.append({
            "encT": encT_b,
            "jw1enc": jw1enc,
            "jw2d": jw2_np,
            "biasu": bias_b,
            "seld": sel,
        })
    return in_maps


def kernel(encoder_states, encoder_states_size, targets, targets_size,
           emb, pw1, pb1, pw2, pb2, jw1, jb1, jw2, jb2):
    if "nc" not in _CACHE:
        _CACHE["nc"] = _build_program()
    nc = _CACHE["nc"]

    in_maps = _make_in_maps(encoder_states, targets, emb, pw1, pb1, pw2, pb2,
                            jw1, jb1, jw2)
    _CACHE["in_maps"] = in_maps
    res = run_bass_kernel_spmd(nc, in_maps, core_ids=list(range(N_CORES)))

    out = np.empty((B, T, U1, V), np.float32)
    for b in range(B):
        out[b] = res.results[b]["out"].reshape(T, U1, V)  # fp16 -> fp32 upcast
    out += np.asarray(jb2, np.float32)  # jb2 epilogue (host)
    # ragged masking (host epilogue)
    tsz = np.asarray(encoder_states_size).astype(np.int64)
    usz = np.asarray(targets_size).astype(np.int64) + 1
    for b in range(B):
        out[b, tsz[b]:, :, :] = 0.0
        out[b, :, usz[b]:, :] = 0.0
    return out


# revision 8
# speedup vs baseline: 3.2943x; 3.2943x over previous
"""FFNN-Transducer joint-lattice kernel for 8 Trainium2 NeuronCores.

The reference zeroes every lattice position with t >= encoder_states_size[b]
or u > targets_size[b], so the kernel only computes the valid ragged region:
the host splits each sample's valid t-range into 128-row blocks, sorts the
blocks by their sample's u-extent, and deals them round-robin across the 8
cores.  Every core runs the same static program over NSLOT block-slots; slot s
uses the compile-time u-extent exts[s] = max u-extent in its group, so the
SPMD program is identical across cores and near-perfectly load balanced.  The
program is specialized (and cached) per (encoder_states_size, targets_size)
tuple at kernel() time; different sizes trigger a rebuild.

Per block the device computes out[t,u,:] = tanh(enc_proj[t,:] + bias[u,:])@jw2
with enc_proj = enc @ jw1[:E] on-device and bias[u,:] from the tiny prediction
network (host, <0.3% of FLOPs). jb2-add, the final masking and the block
scatter are host epilogues.

TRN2 fp32 matmul runs at 1/4 rate, so TensorE-facing tensors are fp16; PSUM
stays fp32; the output is stored fp16 (host upcasts) to halve HBM stores.

Device pipeline per core, per 128-t block-slot s (ext = exts[s]):
  PE:   enc_proj prologue (4-acc matmuls per slot);
        per 4-t chunk one "selection" matmul materializing
        A[j,(t,u<ext)] = enc_proj[t,j] + bias[u,j] in PSUM
        (lhsT = [16 enc rows ; 101 bias rows], rhs = 0/1 selection with
        ext columns per t); per u one [128x128]x[128x88] joint matmul.
  ACT:  batched tanh PSUM->SBUF fp16, contiguous t-major writes (col =
        t_local*ext + u); the joint matmul reads per-u strided lhsT slices
        (measured much cheaper than any scatter-write layout on this HW).
  DVE:  PSUM->SBUF staging evacuation into [t-partition, (u,v)] fp16.
  DMA:  enc_proj bounced through DRAM (2 large descriptors) into the 117-row
        "combined" lhsT tensor (replaces 64 tiny SBUF->SBUF transfers that
        were DMA-issue bound); per-slot stores [128, ext*88] fp16.
"""

import os
import sys

for _p in ("/opt/trn_rl_repo", "/root/.axon_site/_ro/trn_rl_repo"):
    if os.path.isdir(_p) and _p not in sys.path:
        sys.path.append(_p)

import numpy as np

import concourse.bass as bass
import concourse.tile as tile
from concourse import bacc, mybir
from concourse.bass_utils import run_bass_kernel_spmd

# Problem dims (hardcoded per contract)
B, T, E = 8, 1000, 512
U = 100
U1 = U + 1          # 101 joint positions
H, D, P = 2, 256, 256
J, V = 128, 88
BLANK = V - 1
N_CORES = 8

# Device tiling
TB = 128            # t-rows per block-slot
HALF = 8            # t-steps per A-PSUM tile ([128, 1024] = 2 banks)
CH = 4              # t-steps per pre-add matmul chunk
SPAN = 16           # t-steps per combined lhsT slice (K = SPAN + U1 = 117)
UG = 10             # u-steps per M-PSUM tile ([128, 1024] = 2 banks)

F32 = mybir.dt.float32
F16 = mybir.dt.float16

_CACHE = {}


class Plan:
    """Ragged block schedule shared by host prep and program emission."""

    def __init__(self, tsz, usz):
        self.tsz = [int(x) for x in tsz]
        self.uext = [min(int(u) + 1, U1) for u in usz]
        blocks = []  # (uext, sample, t0, n_t)
        for b in range(B):
            t = 0
            while t < self.tsz[b]:
                n_t = min(TB, self.tsz[b] - t)
                blocks.append((self.uext[b], b, t, n_t))
                t += TB
        blocks.sort(key=lambda x: (-x[0], x[1], x[2]))
        self.nslot = (len(blocks) + N_CORES - 1) // N_CORES
        # slot s group = blocks[s*8:(s+1)*8]; pad with dummies (sample -1)
        while len(blocks) < self.nslot * N_CORES:
            blocks.append((0, -1, 0, 0))
        self.exts = []
        self.assign = [[] for _ in range(N_CORES)]  # per core: (sample,t0,n_t)
        for s in range(self.nslot):
            grp = blocks[s * N_CORES:(s + 1) * N_CORES]
            ext = max(5, max(g[0] for g in grp))  # >=5 keeps tiles sane
            self.exts.append(ext)
            for k in range(N_CORES):
                _, smp, t0, n_t = grp[k]
                self.assign[k].append((smp, t0, n_t))
        self.dexts = sorted(set(self.exts), reverse=True)  # distinct exts
        self.selcols = [SPAN * e for e in self.dexts]
        self.seloff = np.cumsum([0] + self.selcols).tolist()
        self.key = (tuple(self.tsz), tuple(self.uext))


def _build_program(reps=1, plan=None):
    if plan is None:
        plan = _CACHE["plan"]
    NS = plan.nslot
    NSPAN = NS * 8
    selw = plan.seloff[-1]

    nc = bacc.Bacc("TRN2", target_bir_lowering=False, debug=False)

    encT = nc.dram_tensor("encT", [E, NS * TB], F16, kind="ExternalInput").ap()
    jw1enc = nc.dram_tensor("jw1enc", [E, J], F16, kind="ExternalInput").ap()
    jw2d = nc.dram_tensor("jw2d", [J, V], F16, kind="ExternalInput").ap()
    biasrep = nc.dram_tensor("biasrep", [U1, NSPAN * J], F16,
                             kind="ExternalInput").ap()
    seld = nc.dram_tensor("seld", [SPAN + U1, selw], F16, kind="ExternalInput").ap()
    epd = nc.dram_tensor("epd", [128, NS * J], F16, kind="Internal").ap()
    out = nc.dram_tensor("out", [NS * TB, U1 * V], F16, kind="ExternalOutput").ap()

    with tile.TileContext(nc) as tc:
        with (
            tc.tile_pool(name="singles", bufs=1) as singles,
            tc.tile_pool(name="hidp", bufs=3) as hidp,
            tc.tile_pool(name="stgp", bufs=2) as stgp,
            tc.tile_pool(name="psA", bufs=2, space="PSUM") as psA,
            tc.tile_pool(name="psM", bufs=2, space="PSUM") as psM,
        ):
            # ---- persistent SBUF tensors ----
            encT_sb = []
            for k in range(4):
                t_ = singles.tile([128, NS * TB], F16, tag=f"encT{k}")
                nc.sync.dma_start(out=t_[:, :], in_=encT[k * 128:(k + 1) * 128, :])
                encT_sb.append(t_)
            jw1_sb = []
            for k in range(4):
                t_ = singles.tile([128, J], F16, tag=f"jw1_{k}")
                nc.sync.dma_start(out=t_[:, :], in_=jw1enc[k * 128:(k + 1) * 128, :])
                jw1_sb.append(t_)
            jw2_sb = singles.tile([J, V], F16, tag="jw2")
            nc.sync.dma_start(out=jw2_sb[:, :], in_=jw2d[:, :])
            sel_sb = singles.tile([SPAN + U1, selw], F16, tag="sel")
            nc.sync.dma_start(out=sel_sb[:, :], in_=seld[:, :])
            # combined lhsT tensors (double-buffered across reps): rows 0:16 =
            # per-span enc_proj slices (refreshed per rep via the DRAM bounce),
            # rows 16:117 = per-slot bias rows (host-replicated input).
            combined = []
            for p_ in range(2):
                t_ = singles.tile([SPAN + U1, NSPAN * J], F16, tag=f"comb{p_}")
                nc.sync.dma_start(out=t_[SPAN:SPAN + U1, :], in_=biasrep[:, :])
                combined.append(t_)
            enc_proj = singles.tile([128, NS * J], F16, tag="encproj")

            for rep in range(reps):
                _emit_rep(nc, plan, hidp, stgp, psA, psM,
                          encT_sb, jw1_sb, jw2_sb, sel_sb,
                          combined[rep % 2], enc_proj, epd, out, rep)

    nc.compile()
    return nc


def _emit_rep(nc, plan, hidp, stgp, psA, psM,
              encT_sb, jw1_sb, jw2_sb, sel_sb, comb, enc_proj, epd, out, rep):
    NS = plan.nslot

    # ---- prologue: enc_proj[t, j] = sum_e enc[t, e] * jw1enc[e, j] ----
    for s in range(NS):
        ep = psA.tile([TB, J], F32, tag="A", name=f"ep{rep}_{s}")
        for k in range(4):
            nc.tensor.matmul(
                ep[:, :],
                encT_sb[k][:, s * TB:(s + 1) * TB],
                jw1_sb[k][:, :],
                start=(k == 0),
                stop=(k == 3),
            )
        nc.vector.tensor_copy(out=enc_proj[:, s * J:(s + 1) * J], in_=ep[:, :])
    # bounce enc_proj through DRAM to land the per-span 16-row slices on
    # partitions 0:16 of `comb` (2 large DMAs instead of 8*NS tiny ones).
    nc.sync.dma_start(out=epd[:, :], in_=enc_proj[:, :])
    # comb[0:16] col layout: (s, sp, j) -> (s*8+sp)*J + j; epd flat offset for
    # (r, s, sp, j) = (sp*16+r)*NS*J + s*128 + j
    src = epd.rearrange("(sp r) (s j) -> r s sp j", sp=8, s=NS)
    dst = comb[0:SPAN, :].rearrange("r (s sp j) -> r s sp j", s=NS, sp=8)
    nc.sync.dma_start(out=dst, in_=src)

    hid_tiles = [None] * NS
    stg_tiles = [None] * NS

    def front(s, step):
        # pre-add matmuls + tanh for the 8-t half (s, step); packed t-major
        ext = plan.exts[s]
        soff = plan.seloff[plan.dexts.index(ext)]
        if step == 0:
            hid_tiles[s] = hidp.tile([128, ext * TB], F16, tag="hid",
                                     name=f"hid{rep}_{s}")
        hid2 = hid_tiles[s]
        sp_g = s * 8 + step // 2  # global span index
        cb_t = comb[:, sp_g * J:(sp_g + 1) * J]
        A = psA.tile([128, 1024], F32, tag="A", name=f"A{rep}_{s}_{step}")
        lh = step % 2  # which pair of chunks within the span
        for c in range(2):
            nc.tensor.matmul(
                A[:, c * 512:c * 512 + CH * ext],
                cb_t,
                sel_sb[:, soff + (lh * 2 + c) * CH * ext:
                       soff + (lh * 2 + c + 1) * CH * ext],
                start=True,
                stop=True,
            )
        # tanh, contiguous t-major write (col = t_local*ext + u)
        base = step * HALF
        nc.scalar.activation(
            out=hid2[:, base * ext:(base + HALF) * ext].rearrange(
                "p (c x) -> p c x", c=2),
            in_=A.rearrange("p (c x) -> p c x", c=2)[:, :, 0:CH * ext],
            func=mybir.ActivationFunctionType.Tanh,
        )

    def back(s, ug):
        # joint matmuls + evacuation for u-group ug of slot s
        ext = plan.exts[s]
        nug = (ext + UG - 1) // UG
        if ug >= nug:
            return
        hid2 = hid_tiles[s]
        if ug == 0:
            stg_tiles[s] = stgp.tile([TB, ext * V], F16, tag="stg",
                                     name=f"stg{rep}_{s}")
        stg = stg_tiles[s]
        u0 = ug * UG
        n_u = UG if ug < nug - 1 else ext - u0
        M = psM.tile([TB, 1024], F32, tag="M", name=f"M{rep}_{s}_{ug}")
        hid_ut = hid2.rearrange("p (t u) -> p u t", u=ext)  # strided per-u lhsT
        for i in range(n_u):
            col = (i // 5) * 512 + (i % 5) * V
            nc.tensor.matmul(
                M[:, col:col + V],
                hid_ut[:, u0 + i, :],
                jw2_sb[:, :],
                start=True,
                stop=True,
            )
        if n_u == UG:
            nc.vector.tensor_copy(
                out=stg[:, u0 * V:(u0 + UG) * V].rearrange("p (bk x) -> p bk x", bk=2),
                in_=M.rearrange("p (bk x) -> p bk x", bk=2)[:, :, 0:5 * V],
            )
        elif n_u > 5:
            nc.vector.tensor_copy(
                out=stg[:, u0 * V:(u0 + 5) * V],
                in_=M[:, 0:5 * V],
            )
            nc.vector.tensor_copy(
                out=stg[:, (u0 + 5) * V:(u0 + n_u) * V],
                in_=M[:, 512:512 + (n_u - 5) * V],
            )
        else:
            nc.vector.tensor_copy(
                out=stg[:, u0 * V:(u0 + n_u) * V],
                in_=M[:, 0:n_u * V],
            )
        if ug == nug - 1:
            nc.sync.dma_start(
                out=out[s * TB:(s + 1) * TB, 0:ext * V],
                in_=stg[:, 0:ext * V],
            )

    # software-pipelined emission: slot s's fronts interleave with s-1's backs
    NUGMAX = (U1 + UG - 1) // UG
    for s in range(NS):
        for step in range(TB // HALF):  # 16
            front(s, step)
            if s >= 1 and step < NUGMAX:
                back(s - 1, step)
    for ug in range(NUGMAX):
        back(NS - 1, ug)


def _host_pred_bias(targets_b, emb, pw1, pb1, pw2, pb2, jw1, jb1):
    """bias[u, j] = (pred @ jw1[E:] + jb1)[u, j] for the 101 joint positions."""
    ext = np.concatenate([np.full(H, BLANK, np.int64), targets_b.astype(np.int64)])
    e = np.concatenate([emb[ext[1:U1 + 1]], emb[ext[0:U1]]], axis=1)  # [101, 512]
    h = np.tanh(e @ pw1 + pb1)
    pred = np.tanh(h @ pw2 + pb2)
    return (pred @ jw1[E:] + jb1).astype(np.float32)  # [101, 128]


def _make_sel(plan):
    """Concatenated per-extent t-major selection matrices."""
    sel = np.zeros((SPAN + U1, plan.seloff[-1]), np.float16)
    for d, ext in enumerate(plan.dexts):
        off = plan.seloff[d]
        for tl in range(SPAN):
            sel[tl, off + tl * ext:off + (tl + 1) * ext] = 1.0
            for u in range(ext):
                sel[SPAN + u, off + tl * ext + u] += 1.0
    return sel


def _make_in_maps(plan, encoder_states, targets, emb, pw1, pb1, pw2, pb2,
                  jw1, jb1, jw2):
    encoder_states = np.asarray(encoder_states, dtype=np.float32)
    jw1 = np.asarray(jw1, dtype=np.float32)
    jw2_np = np.ascontiguousarray(np.asarray(jw2, dtype=np.float32)).astype(np.float16)
    jw1enc = np.ascontiguousarray(jw1[:E]).astype(np.float16)
    sel = _make_sel(plan)

    encT_all = [np.asarray(encoder_states[b].T, np.float16) for b in range(B)]
    bias_all = [
        _host_pred_bias(
            np.asarray(targets[b]), np.asarray(emb, np.float32),
            np.asarray(pw1, np.float32), np.asarray(pb1, np.float32),
            np.asarray(pw2, np.float32), np.asarray(pb2, np.float32),
            jw1, np.asarray(jb1, np.float32),
        ).astype(np.float16)
        for b in range(B)
    ]
    zbias = np.zeros((U1, J), np.float16)

    NS = plan.nslot
    in_maps = []
    for k in range(N_CORES):
        encT_k = np.zeros((E, NS * TB), np.float16)
        brep = np.zeros((U1, NS * 8 * J), np.float16)
        for s, (smp, t0, n_t) in enumerate(plan.assign[k]):
            if smp >= 0:
                encT_k[:, s * TB:s * TB + n_t] = encT_all[smp][:, t0:t0 + n_t]
                bb = bias_all[smp]
            else:
                bb = zbias
            brep[:, s * 8 * J:(s + 1) * 8 * J] = np.tile(bb, (1, 8))
        in_maps.append({
            "encT": encT_k,
            "jw1enc": jw1enc,
            "jw2d": jw2_np,
            "biasrep": brep,
            "seld": sel,
        })
    return in_maps


def kernel(encoder_states, encoder_states_size, targets, targets_size,
           emb, pw1, pb1, pw2, pb2, jw1, jb1, jw2, jb2):
    tsz = np.asarray(encoder_states_size).astype(np.int64)
    usz = np.asarray(targets_size).astype(np.int64)
    plan = Plan(tsz, usz)
    if _CACHE.get("key") != plan.key:
        _CACHE["plan"] = plan
        _CACHE["nc"] = _build_program(reps=1, plan=plan)
        _CACHE["key"] = plan.key
    nc = _CACHE["nc"]
    plan = _CACHE["plan"]

    in_maps = _make_in_maps(plan, encoder_states, targets, emb, pw1, pb1,
                            pw2, pb2, jw1, jb1, jw2)
    _CACHE["in_maps"] = in_maps
    res = run_bass_kernel_spmd(nc, in_maps, core_ids=list(range(N_CORES)))

    jb2 = np.asarray(jb2, np.float32)
    out = np.zeros((B, T, U1, V), np.float32)
    for k in range(N_CORES):
        res_k = res.results[k]["out"]  # [NS*TB, U1*V] f16
        for s, (smp, t0, n_t) in enumerate(plan.assign[k]):
            if smp < 0:
                continue
            ext = plan.exts[s]
            uv = plan.uext[smp]
            blk = res_k[s * TB:s * TB + n_t, 0:ext * V].reshape(n_t, ext, V)
            out[smp, t0:t0 + n_t, 0:uv] = blk[:, 0:uv].astype(np.float32) + jb2
    return out


# revision 13
# speedup vs baseline: 3.7492x; 1.1381x over previous
"""FFNN-Transducer joint-lattice kernel for 8 Trainium2 NeuronCores.

The reference zeroes every lattice position with t >= encoder_states_size[b]
or u > targets_size[b], so the kernel only computes the valid ragged region:
the host splits each sample's valid t-range into 128-row blocks, sorts the
blocks by their sample's u-extent, and deals them round-robin across the 8
cores.  Every core runs the same static program over NSLOT block-slots; slot s
uses the compile-time u-extent exts[s] = max u-extent in its group, so the
SPMD program is identical across cores and near-perfectly load balanced.  The
program is specialized (and cached) per (encoder_states_size, targets_size)
tuple at kernel() time; different sizes trigger a rebuild.

Per block the device computes out[t,u,:] = tanh(enc_proj[t,:] + bias[u,:])@jw2
with enc_proj = enc @ jw1[:E] on-device and bias[u,:] from the tiny prediction
network (host, <0.3% of FLOPs). jb2-add, the final masking and the block
scatter are host epilogues.

TRN2 fp32 matmul runs at 1/4 rate, so TensorE-facing tensors are fp16; PSUM
stays fp32; the output is stored fp16 (host upcasts) to halve HBM stores.

Device pipeline per core, per 128-t block-slot s (ext = exts[s]):
  PE:   enc_proj prologue (4-acc matmuls per slot);
        per 4-t chunk one "selection" matmul materializing
        A[j,(t,u<ext)] = enc_proj[t,j] + bias[u,j] in PSUM
        (lhsT = [16 enc rows ; 101 bias rows], rhs = 0/1 selection with
        ext columns per t); per u one [128x128]x[128x88] joint matmul.
  ACT:  batched tanh PSUM->SBUF fp16, contiguous t-major writes (col =
        t_local*ext + u); the joint matmul reads per-u strided lhsT slices
        (measured much cheaper than any scatter-write layout on this HW).
  DVE:  PSUM->SBUF staging evacuation into [t-partition, (u,v)] fp16.
  DMA:  enc_proj bounced through DRAM (2 large descriptors) into the 117-row
        "combined" lhsT tensor (replaces 64 tiny SBUF->SBUF transfers that
        were DMA-issue bound); per-slot stores [128, ext*88] fp16.
"""

import os
import sys

for _p in ("/opt/trn_rl_repo", "/root/.axon_site/_ro/trn_rl_repo"):
    if os.path.isdir(_p) and _p not in sys.path:
        sys.path.append(_p)

import numpy as np

import concourse.bass as bass
import concourse.tile as tile
from concourse import bacc, mybir
from concourse.bass_utils import run_bass_kernel_spmd

# Problem dims (hardcoded per contract)
B, T, E = 8, 1000, 512
U = 100
U1 = U + 1          # 101 joint positions
H, D, P = 2, 256, 256
J, V = 128, 88
BLANK = V - 1
N_CORES = 8

# Device tiling
TB = 128            # t-rows per block-slot
HALF = 8            # t-steps per A-PSUM tile ([128, 1024] = 2 banks)
CH = 4              # t-steps per pre-add matmul chunk
SPAN = 16           # t-steps per combined lhsT slice (K = SPAN + U1 = 117)
UG = 10             # u-steps per M-PSUM tile ([128, 1024] = 2 banks)

F32 = mybir.dt.float32
F16 = mybir.dt.float16

# tuning knobs (overridable by bench harnesses)
HID_BUFS = 3
STG_BUFS = 3
A12 = True    # True: 12t/3-bank A tiles + 5u/1-bank M tiles
VARCH = False  # True: 8-t chunks for ext<=64 slots (fewer, wider front MMs)
EVAC_ACT = 0   # every Nth evacuation copy goes to ScalarE (0 = never)
SPLIT_BOUNCE = 0  # bounce enc_proj->comb in per-G-slot chunks (0 = whole)
EP_ON_M = False   # prologue PSUM tiles share the M pool instead of A

_CACHE = {}


class Plan:
    """Ragged block schedule shared by host prep and program emission."""

    def __init__(self, tsz, usz, full_u=False, full_t=False):
        self.tsz = [T if full_t else int(x) for x in tsz]
        self.uext = [U1 if full_u else min(int(u) + 1, U1) for u in usz]
        blocks = []  # (uext, sample, t0, n_t)
        for b in range(B):
            t = 0
            while t < self.tsz[b]:
                n_t = min(TB, self.tsz[b] - t)
                blocks.append((self.uext[b], b, t, n_t))
                t += TB
        blocks.sort(key=lambda x: (-x[0], x[1], x[2]))
        self.nslot = (len(blocks) + N_CORES - 1) // N_CORES
        # slot s group = blocks[s*8:(s+1)*8]; pad with dummies (sample -1)
        while len(blocks) < self.nslot * N_CORES:
            blocks.append((0, -1, 0, 0))
        self.exts = []
        self.assign = [[] for _ in range(N_CORES)]  # per core: (sample,t0,n_t)
        for s in range(self.nslot):
            grp = blocks[s * N_CORES:(s + 1) * N_CORES]
            ext = max(5, max(g[0] for g in grp))  # >=5 keeps tiles sane
            self.exts.append(ext)
            for k in range(N_CORES):
                _, smp, t0, n_t = grp[k]
                self.assign[k].append((smp, t0, n_t))
        self.dexts = sorted(set(self.exts), reverse=True)  # distinct exts
        self.selcols = [SPAN * e for e in self.dexts]
        self.seloff = np.cumsum([0] + self.selcols).tolist()
        # packed per-slot output offsets (fp16 elements); row pitch ext*V so
        # every per-slot store is one fully contiguous DMA
        self.offs = np.cumsum([0] + [TB * e * V for e in self.exts]).tolist()
        self.key = (tuple(self.tsz), tuple(self.uext))


def _build_program(reps=1, plan=None):
    if plan is None:
        plan = _CACHE["plan"]
    NS = plan.nslot
    NSPAN = NS * 8
    selw = plan.seloff[-1]

    nc = bacc.Bacc("TRN2", target_bir_lowering=False, debug=False)

    encT = nc.dram_tensor("encT", [E, NS * TB], F16, kind="ExternalInput").ap()
    jw1enc = nc.dram_tensor("jw1enc", [E, J], F16, kind="ExternalInput").ap()
    jw2d = nc.dram_tensor("jw2d", [J, V], F16, kind="ExternalInput").ap()
    biasrep = nc.dram_tensor("biasrep", [U1, NSPAN * J], F16,
                             kind="ExternalInput").ap()
    seld = nc.dram_tensor("seld", [SPAN + U1, selw], F16, kind="ExternalInput").ap()
    epd = nc.dram_tensor("epd", [128, NS * J], F16, kind="Internal").ap()
    out = nc.dram_tensor("out", [1, plan.offs[-1]], F16, kind="ExternalOutput").ap()

    with tile.TileContext(nc) as tc:
        with (
            tc.tile_pool(name="singles", bufs=1) as singles,
            tc.tile_pool(name="hidp", bufs=HID_BUFS) as hidp,
            tc.tile_pool(name="stgp", bufs=STG_BUFS) as stgp,
            tc.tile_pool(name="psA", bufs=2, space="PSUM") as psA,
            tc.tile_pool(name="psM", bufs=2, space="PSUM") as psM,
        ):
            # ---- persistent SBUF tensors ----
            encT_sb = []
            for k in range(4):
                t_ = singles.tile([128, NS * TB], F16, tag=f"encT{k}")
                nc.sync.dma_start(out=t_[:, :], in_=encT[k * 128:(k + 1) * 128, :])
                encT_sb.append(t_)
            jw1_sb = []
            for k in range(4):
                t_ = singles.tile([128, J], F16, tag=f"jw1_{k}")
                nc.sync.dma_start(out=t_[:, :], in_=jw1enc[k * 128:(k + 1) * 128, :])
                jw1_sb.append(t_)
            jw2_sb = singles.tile([J, V], F16, tag="jw2")
            nc.sync.dma_start(out=jw2_sb[:, :], in_=jw2d[:, :])
            sel_sb = singles.tile([SPAN + U1, selw], F16, tag="sel")
            nc.sync.dma_start(out=sel_sb[:, :], in_=seld[:, :])
            # combined lhsT tensors (double-buffered across reps): rows 0:16 =
            # per-span enc_proj slices (refreshed per rep via the DRAM bounce),
            # rows 16:117 = per-slot bias rows (host-replicated input).
            combined = []
            for p_ in range(2):
                t_ = singles.tile([SPAN + U1, NSPAN * J], F16, tag=f"comb{p_}")
                nc.sync.dma_start(out=t_[SPAN:SPAN + U1, :], in_=biasrep[:, :])
                combined.append(t_)
            enc_proj = singles.tile([128, NS * J], F16, tag="encproj")

            for rep in range(reps):
                _emit_rep(nc, plan, hidp, stgp, psA, psM,
                          encT_sb, jw1_sb, jw2_sb, sel_sb,
                          combined[rep % 2], enc_proj, epd, out, rep)

    nc.compile()
    return nc


def _emit_rep(nc, plan, hidp, stgp, psA, psM,
              encT_sb, jw1_sb, jw2_sb, sel_sb, comb, enc_proj, epd, out, rep):
    NS = plan.nslot

    # ---- prologue: enc_proj[t, j] = sum_e enc[t, e] * jw1enc[e, j] ----
    # bounced through DRAM (in per-group chunks) to land the per-span 16-row
    # slices on partitions 0:16 of `comb` without 8*NS tiny SBUF transfers.
    G = SPLIT_BOUNCE if SPLIT_BOUNCE else NS
    for s in range(NS):
        pool, tg = (psM, "M") if EP_ON_M else (psA, "A")
        ep = pool.tile([TB, J], F32, tag=tg, name=f"ep{rep}_{s}")
        for k in range(4):
            nc.tensor.matmul(
                ep[:, :],
                encT_sb[k][:, s * TB:(s + 1) * TB],
                jw1_sb[k][:, :],
                start=(k == 0),
                stop=(k == 3),
            )
        nc.vector.tensor_copy(out=enc_proj[:, s * J:(s + 1) * J], in_=ep[:, :])
        if (s + 1) % G == 0 or s == NS - 1:
            g0 = (s // G) * G
            ng = s - g0 + 1
            nc.sync.dma_start(out=epd[:, g0 * J:(s + 1) * J],
                              in_=enc_proj[:, g0 * J:(s + 1) * J])
            # comb[0:16] col layout: (s, sp, j) -> (s*8+sp)*J + j; epd flat
            # offset for (r, s, sp, j) = (sp*16+r)*NS*J + s*128 + j
            src = epd[:, g0 * J:(s + 1) * J].rearrange(
                "(sp r) (s j) -> r s sp j", sp=8, s=ng)
            dst = comb[0:SPAN, g0 * 8 * J:(s + 1) * 8 * J].rearrange(
                "r (s sp j) -> r s sp j", s=ng, sp=8)
            nc.sync.dma_start(out=dst, in_=src)

    hid_tiles = [None] * NS
    stg_tiles = [None] * NS

    # front tiling per slot: chunk width ch (t per matmul, ch | SPAN and
    # ch*ext <= 512) and list of (t0_local, n_chunks) per A tile
    ABANKS = 3 if A12 else 2

    def ftiling(ext):
        ch = 8 if (VARCH and ext <= 64) else CH
        npt = ABANKS  # chunks per A tile
        tiles = []
        t0 = 0
        while t0 < TB:
            nch = min(npt, (TB - t0) // ch)
            tiles.append((t0, ch, nch))
            t0 += ch * nch
        return tiles

    def front(s, step):
        # pre-add matmuls + tanh for A tile `step` of slot s; packed t-major
        ext = plan.exts[s]
        soff = plan.seloff[plan.dexts.index(ext)]
        if step == 0:
            hid_tiles[s] = hidp.tile([128, ext * TB], F16, tag="hid",
                                     name=f"hid{rep}_{s}")
        hid2 = hid_tiles[s]
        tiles = ftiling(ext)
        if step >= len(tiles):
            return
        t0, ch, nch = tiles[step]
        A = psA.tile([128, ABANKS * 512], F32, tag="A", name=f"A{rep}_{s}_{step}")
        for c in range(nch):
            tl = t0 + ch * c
            sp_g = s * 8 + tl // SPAN
            lhc = (tl % SPAN) // ch
            nc.tensor.matmul(
                A[:, c * 512:c * 512 + ch * ext],
                comb[:, sp_g * J:(sp_g + 1) * J],
                sel_sb[:, soff + lhc * ch * ext:soff + (lhc + 1) * ch * ext],
                start=True,
                stop=True,
            )
        # tanh, contiguous t-major write (col = t_local*ext + u)
        nc.scalar.activation(
            out=hid2[:, t0 * ext:(t0 + ch * nch) * ext].rearrange(
                "p (c x) -> p c x", c=nch),
            in_=A.rearrange("p (c x) -> p c x", c=ABANKS)[:, 0:nch, 0:ch * ext],
            func=mybir.ActivationFunctionType.Tanh,
        )

    MUG = 5 if A12 else UG      # u per M tile
    MBANKS = 1 if A12 else 2

    def back(s, ug):
        # joint matmuls + evacuation for u-group ug of slot s
        ext = plan.exts[s]
        nug = (ext + MUG - 1) // MUG
        if ug >= nug:
            return
        hid2 = hid_tiles[s]
        if ug == 0:
            stg_tiles[s] = stgp.tile([TB, ext * V], F16, tag="stg",
                                     name=f"stg{rep}_{s}")
        stg = stg_tiles[s]
        u0 = ug * MUG
        n_u = MUG if ug < nug - 1 else ext - u0
        M = psM.tile([TB, MBANKS * 512], F32, tag="M", name=f"M{rep}_{s}_{ug}")
        hid_ut = hid2.rearrange("p (t u) -> p u t", u=ext)  # strided per-u lhsT
        for i in range(n_u):
            col = (i // 5) * 512 + (i % 5) * V
            nc.tensor.matmul(
                M[:, col:col + V],
                hid_ut[:, u0 + i, :],
                jw2_sb[:, :],
                start=True,
                stop=True,
            )
        if A12:
            eng = (nc.scalar if (EVAC_ACT and
                                 (s * 32 + ug) % EVAC_ACT == EVAC_ACT - 1)
                   else nc.vector)
            if eng is nc.scalar:
                nc.scalar.copy(out=stg[:, u0 * V:(u0 + n_u) * V],
                               in_=M[:, 0:n_u * V])
            else:
                nc.vector.tensor_copy(
                    out=stg[:, u0 * V:(u0 + n_u) * V],
                    in_=M[:, 0:n_u * V],
                )
        elif n_u == UG:
            nc.vector.tensor_copy(
                out=stg[:, u0 * V:(u0 + UG) * V].rearrange("p (bk x) -> p bk x", bk=2),
                in_=M.rearrange("p (bk x) -> p bk x", bk=2)[:, :, 0:5 * V],
            )
        elif n_u > 5:
            nc.vector.tensor_copy(
                out=stg[:, u0 * V:(u0 + 5) * V],
                in_=M[:, 0:5 * V],
            )
            nc.vector.tensor_copy(
                out=stg[:, (u0 + 5) * V:(u0 + n_u) * V],
                in_=M[:, 512:512 + (n_u - 5) * V],
            )
        else:
            nc.vector.tensor_copy(
                out=stg[:, u0 * V:(u0 + n_u) * V],
                in_=M[:, 0:n_u * V],
            )
        if ug == nug - 1:
            dst = out[0, plan.offs[s]:plan.offs[s + 1]].rearrange(
                "(t x) -> t x", t=TB)
            nc.sync.dma_start(out=dst, in_=stg[:, 0:ext * V])

    # software-pipelined emission: slot s's fronts interleave with s-1's backs
    for s in range(NS):
        nf = len(ftiling(plan.exts[s]))
        nb = 0 if s == 0 else (plan.exts[s - 1] + MUG - 1) // MUG
        bpf = (nb + nf - 1) // nf
        done = 0
        for step in range(nf):
            front(s, step)
            for q in range(bpf):
                if done < nb:
                    back(s - 1, done)
                    done += 1
    nb = (plan.exts[NS - 1] + MUG - 1) // MUG
    for ug in range(nb):
        back(NS - 1, ug)


def _host_pred_bias(targets_b, emb, pw1, pb1, pw2, pb2, jw1, jb1):
    """bias[u, j] = (pred @ jw1[E:] + jb1)[u, j] for the 101 joint positions."""
    ext = np.concatenate([np.full(H, BLANK, np.int64), targets_b.astype(np.int64)])
    e = np.concatenate([emb[ext[1:U1 + 1]], emb[ext[0:U1]]], axis=1)  # [101, 512]
    h = np.tanh(e @ pw1 + pb1)
    pred = np.tanh(h @ pw2 + pb2)
    return (pred @ jw1[E:] + jb1).astype(np.float32)  # [101, 128]


def _make_sel(plan):
    """Concatenated per-extent t-major selection matrices."""
    sel = np.zeros((SPAN + U1, plan.seloff[-1]), np.float16)
    for d, ext in enumerate(plan.dexts):
        off = plan.seloff[d]
        for tl in range(SPAN):
            sel[tl, off + tl * ext:off + (tl + 1) * ext] = 1.0
            for u in range(ext):
                sel[SPAN + u, off + tl * ext + u] += 1.0
    return sel


def _make_in_maps(plan, encoder_states, targets, emb, pw1, pb1, pw2, pb2,
                  jw1, jb1, jw2):
    encoder_states = np.asarray(encoder_states, dtype=np.float32)
    jw1 = np.asarray(jw1, dtype=np.float32)
    jw2_np = np.ascontiguousarray(np.asarray(jw2, dtype=np.float32)).astype(np.float16)
    jw1enc = np.ascontiguousarray(jw1[:E]).astype(np.float16)
    sel = _make_sel(plan)

    encT_all = [np.asarray(encoder_states[b].T, np.float16) for b in range(B)]
    bias_all = [
        _host_pred_bias(
            np.asarray(targets[b]), np.asarray(emb, np.float32),
            np.asarray(pw1, np.float32), np.asarray(pb1, np.float32),
            np.asarray(pw2, np.float32), np.asarray(pb2, np.float32),
            jw1, np.asarray(jb1, np.float32),
        ).astype(np.float16)
        for b in range(B)
    ]
    zbias = np.zeros((U1, J), np.float16)

    NS = plan.nslot
    in_maps = []
    for k in range(N_CORES):
        encT_k = np.zeros((E, NS * TB), np.float16)
        brep = np.zeros((U1, NS * 8 * J), np.float16)
        for s, (smp, t0, n_t) in enumerate(plan.assign[k]):
            if smp >= 0:
                encT_k[:, s * TB:s * TB + n_t] = encT_all[smp][:, t0:t0 + n_t]
                bb = bias_all[smp]
            else:
                bb = zbias
            brep[:, s * 8 * J:(s + 1) * 8 * J] = np.tile(bb, (1, 8))
        in_maps.append({
            "encT": encT_k,
            "jw1enc": jw1enc,
            "jw2d": jw2_np,
            "biasrep": brep,
            "seld": sel,
        })
    return in_maps


def kernel(encoder_states, encoder_states_size, targets, targets_size,
           emb, pw1, pb1, pw2, pb2, jw1, jb1, jw2, jb2):
    tsz = np.asarray(encoder_states_size).astype(np.int64)
    usz = np.asarray(targets_size).astype(np.int64)
    plan = Plan(tsz, usz)
    if _CACHE.get("key") != plan.key:
        _CACHE["plan"] = plan
        _CACHE["nc"] = _build_program(reps=1, plan=plan)
        _CACHE["key"] = plan.key
    nc = _CACHE["nc"]
    plan = _CACHE["plan"]

    in_maps = _make_in_maps(plan, encoder_states, targets, emb, pw1, pb1,
                            pw2, pb2, jw1, jb1, jw2)
    _CACHE["in_maps"] = in_maps
    res = run_bass_kernel_spmd(nc, in_maps, core_ids=list(range(N_CORES)))

    jb2 = np.asarray(jb2, np.float32)
    out = np.zeros((B, T, U1, V), np.float32)
    for k in range(N_CORES):
        res_k = res.results[k]["out"].reshape(-1)  # packed f16
        for s, (smp, t0, n_t) in enumerate(plan.assign[k]):
            if smp < 0:
                continue
            ext = plan.exts[s]
            uv = plan.uext[smp]
            blk = res_k[plan.offs[s]:plan.offs[s + 1]].reshape(TB, ext, V)
            out[smp, t0:t0 + n_t, 0:uv] = (
                blk[0:n_t, 0:uv].astype(np.float32) + jb2)
    return out
